# revision 17
# baseline (speedup 1.0000x reference)
"""Trainium2 Bass kernel for nn_ALAttention (sparse local attention).

Sharding: 64 image rows split across 8 cores (8 query rows each). Each core
gets a host-gathered 16-row key/value slab; border cores (0 and 7) use
custom slab row arrangements with duplicated rows so that every query pair
t (128 queries = 2 image rows) attends only within slab chunks t..t+4
(host-asserted) -- this covers the clamped window/leg geometry at image
borders while keeping a uniform SPMD graph.

Per core: QKV GEMM in bf16 (k bias dropped via softmax shift-invariance;
v bias folded into the proj bias on host; q scale folded into Wq/bq on
host -- all exact), V produced directly in transposed [key, dim] layout by
a GEMM with x as the stationary operand (no PE transposes), then banded
masked attention per head pair: for each of 8 key chunks, scores S^T
[128 keys x (w_j*128) queries] against only the query pairs that attend
that chunk (w_j in 1,2,3,4,4,3,2,1), exp on Act, mask-mult on DVE/Pool,
AV matmul accumulating numerator and (via an augmented ones column in V)
the softmax denominator into PSUM. Normalization: fast reciprocal of the
denominator rows, PE broadcast via a block-diagonal ones matmul, two DVE
multiplies. Finally the proj GEMM with fused bias.

QKV work for batch 1 and proj for batch 0 are interleaved into the
attention pairs as real "filler" PE work to plug pipeline bubbles and keep
the PE p-state high.
"""
import os
import sys
import types
from collections import deque

sys.path.insert(0, "/opt/trn_rl_repo")

import numpy as np
import ml_dtypes

from concourse import bacc, tile, mybir
from concourse import bass_utils
from concourse.bass_utils import run_bass_kernel_spmd

F32 = mybir.dt.float32
BF16 = mybir.dt.bfloat16
AF = mybir.ActivationFunctionType
ALU = mybir.AluOpType

B = 2
C = 384
HH = WW = 64
NCORES = 8
ROWS = 8
SLAB = 16
SCOLS = SLAB * WW      # 1024 slab key positions
QCOLS = ROWS * WW      # 512 queries per core
SCALE = float(64) ** -0.5
PAD = -1

W_J = [1, 2, 3, 4, 4, 3, 2, 1]          # query pairs attending key chunk j
T0_J = [max(0, j - 4) for j in range(8)]  # first attending pair
OFF1 = np.cumsum([0] + [w * 128 for w in W_J]).tolist()  # per-half mask col offsets
MASKW = OFF1[-1]       # 2560 per half

LAST_EXEC_NS = None
LAST_TRACE = None
_NC_CACHE = {}


def _register_ntff_hook():
    if "antenv.axon_hooks" in sys.modules:
        return
    try:
        from trn_agent_boot.trn_boot import _ntff_profile_via_ctypes
        hook = _ntff_profile_via_ctypes("/opt/axon/libaxon_pjrt.so")
    except Exception:
        hook = None
    mod = types.ModuleType("antenv.axon_hooks")
    mod.get_axon_ntff_profile_hook = lambda: hook
    mod.set_axon_ntff_profile_hook = lambda h: None
    sys.modules["antenv.axon_hooks"] = mod
    bass_utils.upload_artifacts = lambda tmpdir: "local://skipped"


def slab_rows_for_core(i):
    if i == 0:
        return [6, 7, 8, 9] + list(range(0, 8)) + [8, 9, 10, 11]
    if i == 7:
        return [52, 53, 54, 55] + list(range(56, 64)) + [55, 56, 57, PAD]
    return list(range(8 * i - 4, 8 * i + 12))


def build_graph():
    nc = bacc.Bacc("TRN2", target_bir_lowering=False, debug=False,
                   num_devices=NCORES)

    xs_e = nc.dram_tensor("xs", [B, C, SCOLS], BF16, kind="ExternalInput").ap()
    # columns host-permuted to [K0 Q0 K1 Q1 K2 Q2 | V(384)]; Q cols pre-scaled
    wqkvT_e = nc.dram_tensor("wqkvT", [C, 3 * C], BF16, kind="ExternalInput").ap()
    bq_e = nc.dram_tensor("bq", [128, 3], F32, kind="ExternalInput").ap()
    wprojT_e = nc.dram_tensor("wprojT", [C, C], BF16, kind="ExternalInput").ap()
    bp_e = nc.dram_tensor("bp", [128, 3], F32, kind="ExternalInput").ap()
    mask_e = nc.dram_tensor("mask", [128, 2, MASKW], BF16,
                            kind="ExternalInput").ap()
    out_e = nc.dram_tensor("out", [B, C, QCOLS], F32, kind="ExternalOutput").ap()

    with tile.TileContext(nc) as tc:
        with (
            tc.tile_pool(name="const", bufs=1) as cpool,
            tc.tile_pool(name="esb", bufs=3) as epool,
            tc.tile_pool(name="sc", bufs=3) as scpool,
            tc.tile_pool(name="osb", bufs=2) as opool,
            tc.tile_pool(name="pmm", bufs=2, space="PSUM") as pmm,
            tc.tile_pool(name="pacc", bufs=2, space="PSUM") as pacc,
        ):
            # ---- constants / inputs ----
            x_sb = [cpool.tile([128, 3, SCOLS], BF16, tag=f"x{b}", name=f"x_sb{b}")
                    for b in range(B)]
            w_sb = cpool.tile([128, 3, 3 * C], BF16, tag="wqkv")
            wp_sb = cpool.tile([128, 3, C], BF16, tag="wproj")
            bq_sb = cpool.tile([128, 3], F32, tag="bq")
            bp_sb = cpool.tile([128, 3], F32, tag="bp")
            mask_sb = cpool.tile([128, 2, MASKW], BF16, tag="mask")
            ones_sb = cpool.tile([1, 64], BF16, tag="ones")

            # DMA queues: sync / scalar / gpsimd round out the early loads
            for kk, q in enumerate((nc.sync, nc.scalar, nc.gpsimd)):
                q.dma_start(x_sb[0][:, kk, :], xs_e[0, 128 * kk:128 * (kk + 1), :])
                q.dma_start(w_sb[:, kk, 0:256],
                            wqkvT_e[128 * kk:128 * (kk + 1), 0:256])
                q.dma_start(w_sb[:, kk, 256:1152],
                            wqkvT_e[128 * kk:128 * (kk + 1), 256:1152])
                q.dma_start(x_sb[1][:, kk, :], xs_e[1, 128 * kk:128 * (kk + 1), :])
            nc.sync.dma_start(bq_sb[:], bq_e[:])
            nc.scalar.dma_start(bp_sb[:], bp_e[:])
            nc.gpsimd.dma_start(mask_sb[:], mask_e[:])
            for kk in range(3):
                nc.sync.dma_start(wp_sb[:, kk, :],
                                  wprojT_e[128 * kk:128 * (kk + 1), :])
            nc.gpsimd.memset(ones_sb[:], 1.0)
            # pre-warm the scalar engine's EXP table
            warm_sb = cpool.tile([1, 1], F32, tag="warm")
            nc.gpsimd.memset(warm_sb[:], 0.0)
            nc.scalar.activation(warm_sb[:], warm_sb[:], AF.Exp)

            # qkv staging in SBUF
            k_sb = [cpool.tile([128, 3, SCOLS], BF16, tag=f"k{b}", name=f"k_sb{b}")
                    for b in range(B)]
            q_sb = [cpool.tile([128, 3, QCOLS], BF16, tag=f"q{b}", name=f"q_sb{b}")
                    for b in range(B)]
            # v: [key-in-chunk, chunk j, half h, pair c, [64 dims|ones]]
            v_sb = [cpool.tile([128, 8, 2, 3, 65], BF16, tag=f"v{b}", name=f"v_sb{b}")
                    for b in range(B)]
            for b in range(B):
                for h in range(2):
                    nc.gpsimd.memset(v_sb[b][:, :, h, :, 64:65], 1.0)
            ocat = [[cpool.tile([128, QCOLS], BF16, tag=f"oc{b}{c}", name=f"ocat{b}{c}")
                     for c in range(3)] for b in range(B)]

            # ---- unit emitters: emit matmuls + copy-out on a given engine ----
            def unit_k(b, c, n, eng):
                def emit():
                    ps = pacc.tile([128, 512], F32, tag="acc",
                                   name=f"kh{b}{c}{n}")
                    for kk in range(3):
                        nc.tensor.matmul(
                            ps[:], w_sb[:, kk, 256 * c:256 * c + 128],
                            x_sb[b][:, kk, 512 * n:512 * (n + 1)],
                            start=(kk == 0), stop=(kk == 2))
                    if eng is nc.scalar:
                        eng.activation(k_sb[b][:, c, 512 * n:512 * (n + 1)],
                                       ps[:], AF.Identity)
                    else:
                        eng.tensor_copy(k_sb[b][:, c, 512 * n:512 * (n + 1)],
                                        ps[:])
                return emit

            def unit_q(b, c):
                def emit():
                    ps = pacc.tile([128, 512], F32, tag="acc",
                                   name=f"qh{b}{c}")
                    for kk in range(3):
                        nc.tensor.matmul(
                            ps[:], w_sb[:, kk, 256 * c + 128:256 * c + 256],
                            x_sb[b][:, kk, 256:768],
                            start=(kk == 0), stop=(kk == 2))
                    nc.vector.tensor_scalar(q_sb[b][:, c, :], ps[:],
                                            bq_sb[:, c:c + 1], None, ALU.add)
                return emit

            def unit_vt(b, pc):
                def emit():
                    ps = pacc.tile([128, 3, 128], F32, tag="acc",
                                   name=f"vt{b}{pc}")
                    for kk in range(3):
                        nc.tensor.matmul(
                            ps[:], x_sb[b][:, kk, 128 * pc:128 * (pc + 1)],
                            w_sb[:, kk, 768:1152],
                            start=(kk == 0), stop=(kk == 2))
                    for h in range(2):
                        nc.vector.tensor_copy(v_sb[b][:, pc, h, :, 0:64],
                                              ps[:, :, 64 * h:64 * h + 64])
                return emit

            def unit_proj(b, m):
                def emit():
                    ps = pacc.tile([128, 512], F32, tag="acc",
                                   name=f"pj{b}{m}")
                    for kk in range(3):
                        nc.tensor.matmul(
                            ps[:], wp_sb[:, kk, 128 * m:128 * (m + 1)],
                            ocat[b][kk][:],
                            start=(kk == 0), stop=(kk == 2))
                    o = opool.tile([128, QCOLS], F32, tag="o", name=f"o{b}{m}")
                    nc.scalar.activation(o[:], ps[:], AF.Identity,
                                         bias=bp_sb[:, m:m + 1], scale=1.0)
                    (nc.sync, nc.scalar, nc.gpsimd)[m].dma_start(
                        out_e[b, 128 * m:128 * (m + 1), :], o[:])
                return emit

            # ---- attention pair ----
            def emit_pair(b, c, fillers):
                ot = [pacc.tile([65, QCOLS], F32, tag="ot", bufs=2,
                                name=f"ot{b}{c}{h}") for h in range(2)]
                # j=3 first: full query width, so its start=True initializes
                # the whole ot bank (partial-width start wipes the full bank)
                for idx, j in enumerate((3, 4, 2, 5, 1, 6, 0, 7)):
                    w = W_J[j]
                    t0 = T0_J[j]
                    wk = w * 128
                    st = pmm.tile([128, 2, 512], F32, tag="st",
                                  name=f"st{b}{c}{j}")
                    for h in range(2):
                        nc.tensor.matmul(
                            st[:, h, 0:wk],
                            k_sb[b][64 * h:64 * h + 64, c, 128 * j:128 * (j + 1)],
                            q_sb[b][64 * h:64 * h + 64, c, 128 * t0:128 * t0 + wk],
                            start=True, stop=True)
                    e = epool.tile([128, 2, 512], BF16, tag="e",
                                   name=f"e{b}{c}{j}")
                    nc.scalar.activation(e[:, :, 0:wk], st[:, :, 0:wk], AF.Exp)
                    meng = nc.gpsimd if j in (0, 7) else nc.vector
                    meng.tensor_tensor(e[:, :, 0:wk], e[:, :, 0:wk],
                                       mask_sb[:, :, OFF1[j]:OFF1[j] + wk],
                                       ALU.mult)
                    for h in range(2):
                        nc.tensor.matmul(
                            ot[h][:, 128 * t0:128 * t0 + wk],
                            v_sb[b][:, j, h, c, :], e[:, h, 0:wk],
                            start=(idx == 0), stop=(idx == 7),
                            skip_group_check=True)
                    if fillers:
                        fillers.popleft()()
                # normalize: denominators sit at partition 64 of ot[h];
                # gather both to partition 0, one reciprocal, broadcast, mult
                srow = scpool.tile([1, 2 * QCOLS], F32, tag="srow",
                                   name=f"srow{b}{c}")
                for h in range(2):
                    nc.vector.tensor_copy(
                        srow[0:1, QCOLS * h:QCOLS * (h + 1)], ot[h][64:65, :])
                rr = scpool.tile([1, 2 * QCOLS], F32, tag="rr",
                                 name=f"rr{b}{c}")
                nc.vector.reciprocal_approx_fast(rr[:], srow[:])
                rb = [scpool.tile([64, QCOLS], F32, tag=f"rb{h}",
                                  name=f"rb{b}{c}{h}") for h in range(2)]
                for h in range(2):
                    nc.gpsimd.partition_broadcast(
                        rb[h][:], rr[0:1, QCOLS * h:QCOLS * (h + 1)])
                for h in range(2):
                    nc.vector.tensor_tensor(
                        ocat[b][c][64 * h:64 * h + 64, :], ot[h][0:64, :],
                        rb[h][:], ALU.mult)

            # ---- program ----
            # upfront: enough QKV for pair (0,0), plus all of V^T
            unit_k(0, 0, 0, nc.vector)()
            unit_k(0, 0, 1, nc.vector)()
            unit_q(0, 0)()
            for pc in range(8):
                unit_vt(0, pc)()
            for pc in range(8):
                unit_vt(1, pc)()

            a, v = nc.scalar, nc.vector
            emit_pair(0, 0, deque([unit_k(0, 1, 0, a), unit_k(0, 1, 1, v),
                                   unit_q(0, 1)]))
            emit_pair(0, 1, deque([unit_k(0, 2, 0, a), unit_k(0, 2, 1, v),
                                   unit_q(0, 2)]))
            emit_pair(0, 2, deque([unit_k(1, 0, 0, a), unit_k(1, 0, 1, v),
                                   unit_q(1, 0)]))
            emit_pair(1, 0, deque([unit_k(1, 1, 0, a), unit_k(1, 1, 1, v),
                                   unit_q(1, 1), unit_proj(0, 0)]))
            emit_pair(1, 1, deque([unit_k(1, 2, 0, a), unit_k(1, 2, 1, v),
                                   unit_q(1, 2), unit_proj(0, 1),
                                   unit_proj(0, 2)]))
            emit_pair(1, 2, deque())
            for m in range(3):
                unit_proj(1, m)()

    nc.compile()
    return nc


def _build_inputs(x, w_qkv, b_qkv, w_proj, b_proj, attn_idx):
    bf = ml_dtypes.bfloat16
    x = np.asarray(x, np.float32)
    w_qkv = np.asarray(w_qkv, np.float32)
    b_qkv = np.asarray(b_qkv, np.float32)
    w_proj = np.asarray(w_proj, np.float32)
    b_proj = np.asarray(b_proj, np.float32)
    attn_idx = np.asarray(attn_idx).astype(np.int64)

    wqkvT = np.ascontiguousarray(w_qkv.T)  # [in_c, out_c]; q 0:384 k 384:768 v 768:
    blocks = []
    for c in range(3):
        blocks.append(wqkvT[:, C + 128 * c:C + 128 * (c + 1)])          # K(c)
        blocks.append(wqkvT[:, 128 * c:128 * (c + 1)] * SCALE)          # Q(c)
    blocks.append(wqkvT[:, 2 * C:])                                     # V
    wperm = np.ascontiguousarray(np.concatenate(blocks, axis=1)).astype(bf)

    bq = np.ascontiguousarray(
        (b_qkv[:C] * SCALE).reshape(3, 128).T).astype(np.float32)
    bp = np.ascontiguousarray(
        (b_proj + w_proj @ b_qkv[2 * C:]).reshape(3, 128).T).astype(np.float32)
    wprojT = np.ascontiguousarray(w_proj.T).astype(bf)

    in_maps = []
    for i in range(NCORES):
        sr = slab_rows_for_core(i)
        slab = np.zeros((B, C, SLAB, WW), np.float32)
        for s, r in enumerate(sr):
            if r != PAD:
                slab[:, :, s, :] = x[:, :, r, :]
        slab = np.ascontiguousarray(slab.reshape(B, C, SCOLS)).astype(bf)

        # per-pair row -> slab slot lookup (first occurrence in the band)
        row2slot = np.full((4, HH), -1, np.int64)
        for t in range(4):
            for s in range(2 * t + 9, 2 * t - 1, -1):
                if sr[s] != PAD:
                    row2slot[t, sr[s]] = s
        q0 = 8 * i * WW
        aidx = attn_idx[q0:q0 + QCOLS]          # [512, 33]
        t_of_q = np.arange(QCOLS) // 128
        ar = aidx // WW
        ac = aidx % WW
        slot = row2slot[t_of_q[:, None], ar]
        assert (slot >= 0).all(), f"core {i}: target row outside band"
        lidx = slot * 64 + ac                    # local key position
        j = lidx // 128
        kin = lidx % 128
        qq = np.repeat(np.arange(QCOLS), aidx.shape[1])
        jf = j.ravel()
        col = (np.asarray(OFF1)[jf]
               + (qq - 128 * np.asarray(T0_J)[jf]))
        mask = np.zeros((128, 2, MASKW), np.float32)
        mask[kin.ravel(), 0, col] = 1.0
        mask[:, 1, :] = mask[:, 0, :]
        # every query must have exactly 33 targets in its band
        assert int(mask[:, 0, :].sum()) == QCOLS * aidx.shape[1], f"core {i}"

        bd = np.zeros((2, 128), np.float32)
        bd[0, 0:64] = 1.0
        bd[1, 64:128] = 1.0
        in_maps.append({
            "xs": slab,
            "wqkvT": wperm,
            "bq": bq,
            "wprojT": wprojT,
            "bp": bp,
            "mask": np.ascontiguousarray(mask).astype(bf),
            "bd": bd.astype(bf),
        })
    return in_maps


def kernel(x, w_qkv, b_qkv, w_proj, b_proj, attn_idx):
    global LAST_EXEC_NS, LAST_TRACE
    _register_ntff_hook()
    if "graph" not in _NC_CACHE:
        _NC_CACHE["graph"] = build_graph()
    nc = _NC_CACHE["graph"]
    in_maps = _build_inputs(x, w_qkv, b_qkv, w_proj, b_proj, attn_idx)
    trace = bool(int(os.environ.get("BASSK_TRACE", "0")))
    res = run_bass_kernel_spmd(nc, in_maps, core_ids=list(range(NCORES)),
                               trace=trace)
    LAST_EXEC_NS = res.exec_time_ns
    if res.instructions_and_trace is not None:
        LAST_TRACE = res.instructions_and_trace[1]
    out = np.empty((B, C, HH, WW), np.float32)
    for i in range(NCORES):
        o = res.results[i]["out"].reshape(B, C, ROWS, WW)
        out[:, :, 8 * i:8 * i + ROWS, :] = o
    return out


# revision 18
# speedup vs baseline: 1.1162x; 1.1162x over previous
"""Trainium2 Bass kernel for nn_ALAttention (sparse local attention).

Sharding: 64 image rows split across 8 cores (8 query rows each). Each core
gets a host-gathered 16-row key/value slab; border cores (0 and 7) use
custom slab row arrangements with duplicated rows so that every query pair
t (128 queries = 2 image rows) attends only within slab chunks t..t+4
(host-asserted) -- this covers the clamped window/leg geometry at image
borders while keeping a uniform SPMD graph.

Per core: QKV GEMM in bf16 (k bias dropped via softmax shift-invariance;
v bias folded into the proj bias on host; q scale folded into Wq/bq on
host -- all exact), V produced directly in transposed [key, dim] layout by
a GEMM with x as the stationary operand (no PE transposes), then banded
masked attention per head pair: for each of 8 key chunks, scores S^T
[128 keys x (w_j*128) queries] against only the query pairs that attend
that chunk (w_j in 1,2,3,4,4,3,2,1), exp on Act, mask-mult on DVE/Pool,
AV matmul accumulating numerator and (via an augmented ones column in V)
the softmax denominator into PSUM. Normalization: fast reciprocal of the
denominator rows, PE broadcast via a block-diagonal ones matmul, two DVE
multiplies. Finally the proj GEMM with fused bias.

QKV work for batch 1 and proj for batch 0 are interleaved into the
attention pairs as real "filler" PE work to plug pipeline bubbles and keep
the PE p-state high.
"""
import os
import sys
import types
from collections import deque

sys.path.insert(0, "/opt/trn_rl_repo")

import numpy as np
import ml_dtypes

from concourse import bacc, tile, mybir
from concourse import bass_utils
from concourse.bass_utils import run_bass_kernel_spmd

F32 = mybir.dt.float32
BF16 = mybir.dt.bfloat16
AF = mybir.ActivationFunctionType
ALU = mybir.AluOpType

B = 2
C = 384
HH = WW = 64
NCORES = 8
ROWS = 8
SLAB = 16
SCOLS = SLAB * WW      # 1024 slab key positions
QCOLS = ROWS * WW      # 512 queries per core
SCALE = float(64) ** -0.5
PAD = -1

W_J = [1, 2, 3, 4, 4, 3, 2, 1]          # query pairs attending key chunk j
T0_J = [max(0, j - 4) for j in range(8)]  # first attending pair
OFF1 = np.cumsum([0] + [w * 128 for w in W_J]).tolist()  # per-half mask col offsets
MASKW = OFF1[-1]       # 2560 per half

LAST_EXEC_NS = None
LAST_TRACE = None
_NC_CACHE = {}


def _register_ntff_hook():
    if "antenv.axon_hooks" in sys.modules:
        return
    try:
        from trn_agent_boot.trn_boot import _ntff_profile_via_ctypes
        hook = _ntff_profile_via_ctypes("/opt/axon/libaxon_pjrt.so")
    except Exception:
        hook = None
    mod = types.ModuleType("antenv.axon_hooks")
    mod.get_axon_ntff_profile_hook = lambda: hook
    mod.set_axon_ntff_profile_hook = lambda h: None
    sys.modules["antenv.axon_hooks"] = mod
    bass_utils.upload_artifacts = lambda tmpdir: "local://skipped"


def slab_rows_for_core(i):
    if i == 0:
        return [6, 7, 8, 9] + list(range(0, 8)) + [8, 9, 10, 11]
    if i == 7:
        return [52, 53, 54, 55] + list(range(56, 64)) + [55, 56, 57, PAD]
    return list(range(8 * i - 4, 8 * i + 12))


def build_graph():
    nc = bacc.Bacc("TRN2", target_bir_lowering=False, debug=False,
                   num_devices=NCORES)

    xs_e = nc.dram_tensor("xs", [B, C, SCOLS], BF16, kind="ExternalInput").ap()
    # columns host-permuted to [K0 Q0 K1 Q1 K2 Q2 | V(384)]; Q cols pre-scaled
    wqkvT_e = nc.dram_tensor("wqkvT", [C, 3 * C], BF16, kind="ExternalInput").ap()
    bq_e = nc.dram_tensor("bq", [128, 3], F32, kind="ExternalInput").ap()
    wprojT_e = nc.dram_tensor("wprojT", [C, C], BF16, kind="ExternalInput").ap()
    bp_e = nc.dram_tensor("bp", [128, 3], F32, kind="ExternalInput").ap()
    mask_e = nc.dram_tensor("mask", [128, 2, MASKW], BF16,
                            kind="ExternalInput").ap()
    out_e = nc.dram_tensor("out", [B, C, QCOLS], F32, kind="ExternalOutput").ap()

    with tile.TileContext(nc) as tc:
        with (
            tc.tile_pool(name="const", bufs=1) as cpool,
            tc.tile_pool(name="esb", bufs=3) as epool,
            tc.tile_pool(name="sc", bufs=3) as scpool,
            tc.tile_pool(name="osb", bufs=2) as opool,
            tc.tile_pool(name="pmm", bufs=2, space="PSUM") as pmm,
            tc.tile_pool(name="pacc", bufs=2, space="PSUM") as pacc,
        ):
            # ---- constants / inputs ----
            x_sb = [cpool.tile([128, 3, SCOLS], BF16, tag=f"x{b}", name=f"x_sb{b}")
                    for b in range(B)]
            w_sb = cpool.tile([128, 3, 3 * C], BF16, tag="wqkv")
            wp_sb = cpool.tile([128, 3, C], BF16, tag="wproj")
            bq_sb = cpool.tile([128, 3], F32, tag="bq")
            bp_sb = cpool.tile([128, 3], F32, tag="bp")
            mask_sb = cpool.tile([128, 2, MASKW], BF16, tag="mask")
            ones_sb = cpool.tile([1, 64], BF16, tag="ones")

            # DMA queues: sync / scalar / gpsimd round out the early loads
            for kk, q in enumerate((nc.sync, nc.scalar, nc.gpsimd)):
                q.dma_start(x_sb[0][:, kk, :], xs_e[0, 128 * kk:128 * (kk + 1), :])
                q.dma_start(w_sb[:, kk, 0:256],
                            wqkvT_e[128 * kk:128 * (kk + 1), 0:256])
                q.dma_start(w_sb[:, kk, 256:1152],
                            wqkvT_e[128 * kk:128 * (kk + 1), 256:1152])
                q.dma_start(x_sb[1][:, kk, :], xs_e[1, 128 * kk:128 * (kk + 1), :])
            nc.sync.dma_start(bq_sb[:], bq_e[:])
            nc.scalar.dma_start(bp_sb[:], bp_e[:])
            nc.gpsimd.dma_start(mask_sb[:], mask_e[:])
            for kk in range(3):
                nc.sync.dma_start(wp_sb[:, kk, :],
                                  wprojT_e[128 * kk:128 * (kk + 1), :])
            nc.gpsimd.memset(ones_sb[:], 1.0)
            # pre-warm the scalar engine's EXP table
            warm_sb = cpool.tile([1, 1], F32, tag="warm")
            nc.gpsimd.memset(warm_sb[:], 0.0)
            nc.scalar.activation(warm_sb[:], warm_sb[:], AF.Exp)

            # qkv staging in SBUF
            k_sb = [cpool.tile([128, 3, SCOLS], BF16, tag=f"k{b}", name=f"k_sb{b}")
                    for b in range(B)]
            q_sb = [cpool.tile([128, 3, QCOLS], BF16, tag=f"q{b}", name=f"q_sb{b}")
                    for b in range(B)]
            # v: [key-in-chunk, chunk j, half h, pair c, [64 dims|ones]]
            v_sb = [cpool.tile([128, 8, 2, 3, 65], BF16, tag=f"v{b}", name=f"v_sb{b}")
                    for b in range(B)]
            for b in range(B):
                for h in range(2):
                    nc.gpsimd.memset(v_sb[b][:, :, h, :, 64:65], 1.0)
            ocat = [[cpool.tile([128, QCOLS], BF16, tag=f"oc{b}{c}", name=f"ocat{b}{c}")
                     for c in range(3)] for b in range(B)]

            # ---- unit emitters: emit matmuls + copy-out on a given engine ----
            def unit_k(b, c, n, eng):
                def emit():
                    ps = pmm.tile([128, 512], F32, tag="st",
                                  name=f"kh{b}{c}{n}")
                    for kk in range(3):
                        nc.tensor.matmul(
                            ps[:], w_sb[:, kk, 256 * c:256 * c + 128],
                            x_sb[b][:, kk, 512 * n:512 * (n + 1)],
                            start=(kk == 0), stop=(kk == 2))
                    if eng is nc.scalar:
                        eng.activation(k_sb[b][:, c, 512 * n:512 * (n + 1)],
                                       ps[:], AF.Identity)
                    else:
                        eng.tensor_copy(k_sb[b][:, c, 512 * n:512 * (n + 1)],
                                        ps[:])
                return emit

            def unit_q(b, c):
                def emit():
                    ps = pmm.tile([128, 512], F32, tag="st",
                                  name=f"qh{b}{c}")
                    for kk in range(3):
                        nc.tensor.matmul(
                            ps[:], w_sb[:, kk, 256 * c + 128:256 * c + 256],
                            x_sb[b][:, kk, 256:768],
                            start=(kk == 0), stop=(kk == 2))
                    nc.vector.tensor_scalar(q_sb[b][:, c, :], ps[:],
                                            bq_sb[:, c:c + 1], None, ALU.add)
                return emit

            def unit_vt(b, pc):
                def emit():
                    ps = pmm.tile([128, 3, 128], F32, tag="st",
                                  name=f"vt{b}{pc}")
                    for kk in range(3):
                        nc.tensor.matmul(
                            ps[:], x_sb[b][:, kk, 128 * pc:128 * (pc + 1)],
                            w_sb[:, kk, 768:1152],
                            start=(kk == 0), stop=(kk == 2))
                    for h in range(2):
                        nc.vector.tensor_copy(v_sb[b][:, pc, h, :, 0:64],
                                              ps[:, :, 64 * h:64 * h + 64])
                return emit

            def unit_proj(b, m):
                def emit():
                    ps = pmm.tile([128, 512], F32, tag="st",
                                  name=f"pj{b}{m}")
                    for kk in range(3):
                        nc.tensor.matmul(
                            ps[:], wp_sb[:, kk, 128 * m:128 * (m + 1)],
                            ocat[b][kk][:],
                            start=(kk == 0), stop=(kk == 2))
                    o = opool.tile([128, QCOLS], F32, tag="o", name=f"o{b}{m}")
                    nc.scalar.activation(o[:], ps[:], AF.Identity,
                                         bias=bp_sb[:, m:m + 1], scale=1.0)
                    (nc.sync, nc.scalar, nc.gpsimd)[m].dma_start(
                        out_e[b, 128 * m:128 * (m + 1), :], o[:])
                return emit

            # ---- attention: pair-granular software pipeline ----
            # scores/exp/mask of pair n+1 run while the AV matmuls of pair n
            # (reading SBUF e tiles) accumulate; norm overlaps via ot bufs=4
            JORD = (3, 4, 2, 5, 1, 6, 0, 7)

            def scores_stream(b, c, es):
                for j in JORD:
                    w = W_J[j]
                    t0 = T0_J[j]
                    wk = w * 128
                    st = pmm.tile([128, 2, 512], F32, tag="st",
                                  name=f"st{b}{c}{j}")
                    for h in range(2):
                        nc.tensor.matmul(
                            st[:, h, 0:wk],
                            k_sb[b][64 * h:64 * h + 64, c, 128 * j:128 * (j + 1)],
                            q_sb[b][64 * h:64 * h + 64, c, 128 * t0:128 * t0 + wk],
                            start=True, stop=True)
                    e = epool.tile([128, 2, wk], BF16, tag=f"e{j}", bufs=2,
                                   name=f"e{b}{c}{j}")
                    nc.scalar.activation(e[:], st[:, :, 0:wk], AF.Exp)
                    meng = nc.gpsimd if j in (0, 7) else nc.vector
                    meng.tensor_tensor(e[:], e[:],
                                       mask_sb[:, :, OFF1[j]:OFF1[j] + wk],
                                       ALU.mult)
                    es.append(e)
                    yield

            def av_stream(b, c, es):
                ot = [pacc.tile([65, QCOLS], F32, tag="ot", bufs=4,
                                name=f"ot{b}{c}{h}") for h in range(2)]
                for idx, j in enumerate(JORD):
                    t0 = T0_J[j]
                    wk = W_J[j] * 128
                    for h in range(2):
                        nc.tensor.matmul(
                            ot[h][:, 128 * t0:128 * t0 + wk],
                            v_sb[b][:, j, h, c, :], es[idx][:, h, :],
                            start=(idx == 0), stop=(idx == 7),
                            skip_group_check=True)
                    yield
                # normalize: denominators sit at partition 64 of ot[h]
                srow = scpool.tile([1, 2 * QCOLS], F32, tag="srow",
                                   name=f"srow{b}{c}")
                for h in range(2):
                    nc.vector.tensor_copy(
                        srow[0:1, QCOLS * h:QCOLS * (h + 1)], ot[h][64:65, :])
                rr = scpool.tile([1, 2 * QCOLS], F32, tag="rr",
                                 name=f"rr{b}{c}")
                nc.vector.reciprocal_approx_fast(rr[:], srow[:])
                rb = [scpool.tile([64, QCOLS], F32, tag=f"rb{h}",
                                  name=f"rb{b}{c}{h}") for h in range(2)]
                for h in range(2):
                    nc.gpsimd.partition_broadcast(
                        rb[h][:], rr[0:1, QCOLS * h:QCOLS * (h + 1)])
                for h in range(2):
                    nc.vector.tensor_tensor(
                        ocat[b][c][64 * h:64 * h + 64, :], ot[h][0:64, :],
                        rb[h][:], ALU.mult)

            # ---- program ----
            unit_k(0, 0, 0, nc.vector)()
            unit_k(0, 0, 1, nc.vector)()
            unit_q(0, 0)()
            for pc in range(8):
                unit_vt(0, pc)()
            for pc in range(8):
                unit_vt(1, pc)()

            a, v = nc.scalar, nc.vector
            pairs = [(0, 0), (0, 1), (0, 2), (1, 0), (1, 1), (1, 2)]
            fillers = [
                deque([unit_k(0, 1, 0, a), unit_k(0, 1, 1, v), unit_q(0, 1)]),
                deque([unit_k(0, 2, 0, a), unit_k(0, 2, 1, v), unit_q(0, 2)]),
                deque([unit_k(1, 0, 0, a), unit_k(1, 0, 1, v), unit_q(1, 0)]),
                deque([unit_k(1, 1, 0, a), unit_k(1, 1, 1, v), unit_q(1, 1)]),
                deque([unit_k(1, 2, 0, a), unit_k(1, 2, 1, v), unit_q(1, 2),
                       unit_proj(0, 0)]),
                deque([unit_proj(0, 1), unit_proj(0, 2)]),
            ]
            prev_av = None
            for (b, c), fl in zip(pairs, fillers):
                es = []
                sg = scores_stream(b, c, es)
                for _ in range(8):
                    next(sg)
                    if prev_av is not None:
                        next(prev_av, None)
                    if fl:
                        fl.popleft()()
                if prev_av is not None:
                    for _ in prev_av:
                        pass
                prev_av = av_stream(b, c, es)
            for _ in prev_av:
                pass
            for m in range(3):
                unit_proj(1, m)()

    nc.compile()
    return nc


def _build_inputs(x, w_qkv, b_qkv, w_proj, b_proj, attn_idx):
    bf = ml_dtypes.bfloat16
    x = np.asarray(x, np.float32)
    w_qkv = np.asarray(w_qkv, np.float32)
    b_qkv = np.asarray(b_qkv, np.float32)
    w_proj = np.asarray(w_proj, np.float32)
    b_proj = np.asarray(b_proj, np.float32)
    attn_idx = np.asarray(attn_idx).astype(np.int64)

    wqkvT = np.ascontiguousarray(w_qkv.T)  # [in_c, out_c]; q 0:384 k 384:768 v 768:
    blocks = []
    for c in range(3):
        blocks.append(wqkvT[:, C + 128 * c:C + 128 * (c + 1)])          # K(c)
        blocks.append(wqkvT[:, 128 * c:128 * (c + 1)] * SCALE)          # Q(c)
    blocks.append(wqkvT[:, 2 * C:])                                     # V
    wperm = np.ascontiguousarray(np.concatenate(blocks, axis=1)).astype(bf)

    bq = np.ascontiguousarray(
        (b_qkv[:C] * SCALE).reshape(3, 128).T).astype(np.float32)
    bp = np.ascontiguousarray(
        (b_proj + w_proj @ b_qkv[2 * C:]).reshape(3, 128).T).astype(np.float32)
    wprojT = np.ascontiguousarray(w_proj.T).astype(bf)

    in_maps = []
    for i in range(NCORES):
        sr = slab_rows_for_core(i)
        slab = np.zeros((B, C, SLAB, WW), np.float32)
        for s, r in enumerate(sr):
            if r != PAD:
                slab[:, :, s, :] = x[:, :, r, :]
        slab = np.ascontiguousarray(slab.reshape(B, C, SCOLS)).astype(bf)

        # per-pair row -> slab slot lookup (first occurrence in the band)
        row2slot = np.full((4, HH), -1, np.int64)
        for t in range(4):
            for s in range(2 * t + 9, 2 * t - 1, -1):
                if sr[s] != PAD:
                    row2slot[t, sr[s]] = s
        q0 = 8 * i * WW
        aidx = attn_idx[q0:q0 + QCOLS]          # [512, 33]
        t_of_q = np.arange(QCOLS) // 128
        ar = aidx // WW
        ac = aidx % WW
        slot = row2slot[t_of_q[:, None], ar]
        assert (slot >= 0).all(), f"core {i}: target row outside band"
        lidx = slot * 64 + ac                    # local key position
        j = lidx // 128
        kin = lidx % 128
        qq = np.repeat(np.arange(QCOLS), aidx.shape[1])
        jf = j.ravel()
        col = (np.asarray(OFF1)[jf]
               + (qq - 128 * np.asarray(T0_J)[jf]))
        mask = np.zeros((128, 2, MASKW), np.float32)
        mask[kin.ravel(), 0, col] = 1.0
        mask[:, 1, :] = mask[:, 0, :]
        # every query must have exactly 33 targets in its band
        assert int(mask[:, 0, :].sum()) == QCOLS * aidx.shape[1], f"core {i}"

        bd = np.zeros((2, 128), np.float32)
        bd[0, 0:64] = 1.0
        bd[1, 64:128] = 1.0
        in_maps.append({
            "xs": slab,
            "wqkvT": wperm,
            "bq": bq,
            "wprojT": wprojT,
            "bp": bp,
            "mask": np.ascontiguousarray(mask).astype(bf),
            "bd": bd.astype(bf),
        })
    return in_maps


def kernel(x, w_qkv, b_qkv, w_proj, b_proj, attn_idx):
    global LAST_EXEC_NS, LAST_TRACE
    _register_ntff_hook()
    if "graph" not in _NC_CACHE:
        _NC_CACHE["graph"] = build_graph()
    nc = _NC_CACHE["graph"]
    in_maps = _build_inputs(x, w_qkv, b_qkv, w_proj, b_proj, attn_idx)
    trace = bool(int(os.environ.get("BASSK_TRACE", "0")))
    res = run_bass_kernel_spmd(nc, in_maps, core_ids=list(range(NCORES)),
                               trace=trace)
    LAST_EXEC_NS = res.exec_time_ns
    if res.instructions_and_trace is not None:
        LAST_TRACE = res.instructions_and_trace[1]
    out = np.empty((B, C, HH, WW), np.float32)
    for i in range(NCORES):
        o = res.results[i]["out"].reshape(B, C, ROWS, WW)
        out[:, :, 8 * i:8 * i + ROWS, :] = o
    return out


# revision 19
# speedup vs baseline: 1.1727x; 1.0507x over previous
"""Trainium2 Bass kernel for nn_ALAttention (sparse local attention).

Sharding: 64 image rows split across 8 cores (8 query rows each). Each core
gets a host-gathered 16-row key/value slab; border cores (0 and 7) use
custom slab row arrangements with duplicated rows so that every query pair
t (128 queries = 2 image rows) attends only within slab chunks t..t+4
(host-asserted) -- this covers the clamped window/leg geometry at image
borders while keeping a uniform SPMD graph.

Per core: QKV GEMM in bf16 (k bias dropped via softmax shift-invariance;
v bias folded into the proj bias on host; q scale folded into Wq/bq on
host -- all exact), V produced directly in transposed [key, dim] layout by
a GEMM with x as the stationary operand (no PE transposes), then banded
masked attention per head pair: for each of 8 key chunks, scores S^T
[128 keys x (w_j*128) queries] against only the query pairs that attend
that chunk (w_j in 1,2,3,4,4,3,2,1), exp on Act, mask-mult on DVE/Pool,
AV matmul accumulating numerator and (via an augmented ones column in V)
the softmax denominator into PSUM. Normalization: fast reciprocal of the
denominator rows, PE broadcast via a block-diagonal ones matmul, two DVE
multiplies. Finally the proj GEMM with fused bias.

QKV work for batch 1 and proj for batch 0 are interleaved into the
attention pairs as real "filler" PE work to plug pipeline bubbles and keep
the PE p-state high.
"""
import os
import sys
import types
from collections import deque

sys.path.insert(0, "/opt/trn_rl_repo")

import numpy as np
import ml_dtypes

from concourse import bacc, tile, mybir
from concourse import bass_utils
from concourse.bass_utils import run_bass_kernel_spmd

F32 = mybir.dt.float32
BF16 = mybir.dt.bfloat16
AF = mybir.ActivationFunctionType
ALU = mybir.AluOpType

B = 2
C = 384
HH = WW = 64
NCORES = 8
ROWS = 8
SLAB = 16
SCOLS = SLAB * WW      # 1024 slab key positions
QCOLS = ROWS * WW      # 512 queries per core
SCALE = float(64) ** -0.5
PAD = -1

W_J = [1, 2, 3, 4, 4, 3, 2, 1]          # query pairs attending key chunk j
T0_J = [max(0, j - 4) for j in range(8)]  # first attending pair
OFF1 = np.cumsum([0] + [w * 128 for w in W_J]).tolist()  # per-half mask col offsets
MASKW = OFF1[-1]       # 2560 per half

LAST_EXEC_NS = None
LAST_TRACE = None
_NC_CACHE = {}


def _register_ntff_hook():
    if "antenv.axon_hooks" in sys.modules:
        return
    try:
        from trn_agent_boot.trn_boot import _ntff_profile_via_ctypes
        hook = _ntff_profile_via_ctypes("/opt/axon/libaxon_pjrt.so")
    except Exception:
        hook = None
    mod = types.ModuleType("antenv.axon_hooks")
    mod.get_axon_ntff_profile_hook = lambda: hook
    mod.set_axon_ntff_profile_hook = lambda h: None
    sys.modules["antenv.axon_hooks"] = mod
    bass_utils.upload_artifacts = lambda tmpdir: "local://skipped"


def slab_rows_for_core(i):
    if i == 0:
        return [6, 7, 8, 9] + list(range(0, 8)) + [8, 9, 10, 11]
    if i == 7:
        return [52, 53, 54, 55] + list(range(56, 64)) + [55, 56, 57, PAD]
    return list(range(8 * i - 4, 8 * i + 12))


def build_graph():
    nc = bacc.Bacc("TRN2", target_bir_lowering=False, debug=False,
                   num_devices=NCORES)

    xs_e = nc.dram_tensor("xs", [B, C, SCOLS], BF16, kind="ExternalInput").ap()
    # columns host-permuted to [K0 Q0 K1 Q1 K2 Q2 | V(384)]; Q cols pre-scaled
    wqkvT_e = nc.dram_tensor("wqkvT", [C, 3 * C], BF16, kind="ExternalInput").ap()
    bq_e = nc.dram_tensor("bq", [128, 3], F32, kind="ExternalInput").ap()
    wprojT_e = nc.dram_tensor("wprojT", [C, C], BF16, kind="ExternalInput").ap()
    bp_e = nc.dram_tensor("bp", [128, 3], F32, kind="ExternalInput").ap()
    mask_e = nc.dram_tensor("mask", [128, 2, MASKW], BF16,
                            kind="ExternalInput").ap()
    out_e = nc.dram_tensor("out", [B, C, QCOLS], F32, kind="ExternalOutput").ap()

    with tile.TileContext(nc) as tc:
        with (
            tc.tile_pool(name="const", bufs=1) as cpool,
            tc.tile_pool(name="esb", bufs=3) as epool,
            tc.tile_pool(name="sc", bufs=3) as scpool,
            tc.tile_pool(name="osb", bufs=2) as opool,
            tc.tile_pool(name="pmm", bufs=2, space="PSUM") as pmm,
            tc.tile_pool(name="pacc", bufs=2, space="PSUM") as pacc,
        ):
            # ---- constants / inputs ----
            x_sb = [cpool.tile([128, 3, SCOLS], BF16, tag=f"x{b}", name=f"x_sb{b}")
                    for b in range(B)]
            w_sb = cpool.tile([128, 3, 3 * C], BF16, tag="wqkv")
            wp_sb = cpool.tile([128, 3, C], BF16, tag="wproj")
            bq_sb = cpool.tile([128, 3], F32, tag="bq")
            bp_sb = cpool.tile([128, 3], F32, tag="bp")
            mask_sb = cpool.tile([128, 2, MASKW], BF16, tag="mask")
            ones_sb = cpool.tile([1, 64], BF16, tag="ones")

            # DMA queues: sync / scalar / gpsimd round out the early loads
            for kk, q in enumerate((nc.sync, nc.scalar, nc.gpsimd)):
                q.dma_start(x_sb[0][:, kk, :], xs_e[0, 128 * kk:128 * (kk + 1), :])
                q.dma_start(w_sb[:, kk, 0:256],
                            wqkvT_e[128 * kk:128 * (kk + 1), 0:256])
                q.dma_start(w_sb[:, kk, 256:1152],
                            wqkvT_e[128 * kk:128 * (kk + 1), 256:1152])
                q.dma_start(x_sb[1][:, kk, :], xs_e[1, 128 * kk:128 * (kk + 1), :])
            nc.sync.dma_start(bq_sb[:], bq_e[:])
            nc.scalar.dma_start(bp_sb[:], bp_e[:])
            nc.gpsimd.dma_start(mask_sb[:], mask_e[:])
            for kk in range(3):
                nc.sync.dma_start(wp_sb[:, kk, :],
                                  wprojT_e[128 * kk:128 * (kk + 1), :])
            nc.gpsimd.memset(ones_sb[:], 1.0)
            # pre-warm the scalar engine's EXP table
            warm_sb = cpool.tile([1, 1], F32, tag="warm")
            nc.gpsimd.memset(warm_sb[:], 0.0)
            nc.scalar.activation(warm_sb[:], warm_sb[:], AF.Exp)

            # qkv staging in SBUF
            k_sb = [cpool.tile([128, 3, SCOLS], BF16, tag=f"k{b}", name=f"k_sb{b}")
                    for b in range(B)]
            q_sb = [cpool.tile([128, 3, QCOLS], BF16, tag=f"q{b}", name=f"q_sb{b}")
                    for b in range(B)]
            # v: [key-in-chunk, chunk j, half h, pair c, [64 dims|ones]]
            v_sb = [cpool.tile([128, 8, 2, 3, 65], BF16, tag=f"v{b}", name=f"v_sb{b}")
                    for b in range(B)]
            for b in range(B):
                for h in range(2):
                    nc.gpsimd.memset(v_sb[b][:, :, h, :, 64:65], 1.0)
            ocat = [[cpool.tile([128, QCOLS], BF16, tag=f"oc{b}{c}", name=f"ocat{b}{c}")
                     for c in range(3)] for b in range(B)]

            # ---- unit emitters: emit matmuls + copy-out on a given engine ----
            def unit_k(b, c, n, eng):
                def emit():
                    ps = pmm.tile([128, 512], F32, tag="st",
                                  name=f"kh{b}{c}{n}")
                    for kk in range(3):
                        nc.tensor.matmul(
                            ps[:], w_sb[:, kk, 256 * c:256 * c + 128],
                            x_sb[b][:, kk, 512 * n:512 * (n + 1)],
                            start=(kk == 0), stop=(kk == 2))
                    if eng is nc.scalar:
                        eng.activation(k_sb[b][:, c, 512 * n:512 * (n + 1)],
                                       ps[:], AF.Identity)
                    else:
                        eng.tensor_copy(k_sb[b][:, c, 512 * n:512 * (n + 1)],
                                        ps[:])
                return emit

            def unit_q(b, c):
                def emit():
                    ps = pmm.tile([128, 512], F32, tag="st",
                                  name=f"qh{b}{c}")
                    for kk in range(3):
                        nc.tensor.matmul(
                            ps[:], w_sb[:, kk, 256 * c + 128:256 * c + 256],
                            x_sb[b][:, kk, 256:768],
                            start=(kk == 0), stop=(kk == 2))
                    nc.vector.tensor_scalar(q_sb[b][:, c, :], ps[:],
                                            bq_sb[:, c:c + 1], None, ALU.add)
                return emit

            def unit_vt(b, pc, pool_tag="st"):
                def emit():
                    pool = pmm if pool_tag == "st" else pacc
                    ps = pool.tile([128, 3, 128], F32, tag=pool_tag,
                                   name=f"vt{b}{pc}", bufs=None
                                   if pool_tag == "st" else 4)
                    for kk in range(3):
                        nc.tensor.matmul(
                            ps[:], x_sb[b][:, kk, 128 * pc:128 * (pc + 1)],
                            w_sb[:, kk, 768:1152],
                            start=(kk == 0), stop=(kk == 2))
                    for h in range(2):
                        nc.vector.tensor_copy(v_sb[b][:, pc, h, :, 0:64],
                                              ps[:, :, 64 * h:64 * h + 64])
                return emit

            def unit_proj(b, m, pool_tag="st"):
                def emit():
                    pool = pmm if pool_tag == "st" else pacc
                    ps = pool.tile([128, 512], F32, tag=pool_tag,
                                   name=f"pj{b}{m}", bufs=None
                                   if pool_tag == "st" else 4)
                    for kk in range(3):
                        nc.tensor.matmul(
                            ps[:], wp_sb[:, kk, 128 * m:128 * (m + 1)],
                            ocat[b][kk][:],
                            start=(kk == 0), stop=(kk == 2))
                    o = opool.tile([128, QCOLS], F32, tag="o", name=f"o{b}{m}")
                    nc.scalar.activation(o[:], ps[:], AF.Identity,
                                         bias=bp_sb[:, m:m + 1], scale=1.0)
                    (nc.sync, nc.scalar, nc.gpsimd)[m].dma_start(
                        out_e[b, 128 * m:128 * (m + 1), :], o[:])
                return emit

            # ---- attention: pair-granular software pipeline ----
            # scores/exp/mask of pair n+1 run while the AV matmuls of pair n
            # (reading SBUF e tiles) accumulate; norm overlaps via ot bufs=4
            JORD = (3, 4, 2, 5, 1, 6, 0, 7)

            def scores_stream(b, c, es):
                for j in JORD:
                    w = W_J[j]
                    t0 = T0_J[j]
                    wk = w * 128
                    st = pmm.tile([128, 2, 512], F32, tag="st",
                                  name=f"st{b}{c}{j}")
                    for h in range(2):
                        nc.tensor.matmul(
                            st[:, h, 0:wk],
                            k_sb[b][64 * h:64 * h + 64, c, 128 * j:128 * (j + 1)],
                            q_sb[b][64 * h:64 * h + 64, c, 128 * t0:128 * t0 + wk],
                            start=True, stop=True)
                    e = epool.tile([128, 2, wk], BF16, tag=f"e{j}", bufs=2,
                                   name=f"e{b}{c}{j}")
                    nc.scalar.activation(e[:], st[:, :, 0:wk], AF.Exp)
                    meng = nc.gpsimd if j in (0, 7) else nc.vector
                    meng.tensor_tensor(e[:], e[:],
                                       mask_sb[:, :, OFF1[j]:OFF1[j] + wk],
                                       ALU.mult)
                    es.append(e)
                    yield

            def av_stream(b, c, es):
                ot = [pacc.tile([65, QCOLS], F32, tag="ot", bufs=4,
                                name=f"ot{b}{c}{h}") for h in range(2)]
                for idx, j in enumerate(JORD):
                    t0 = T0_J[j]
                    wk = W_J[j] * 128
                    for h in range(2):
                        nc.tensor.matmul(
                            ot[h][:, 128 * t0:128 * t0 + wk],
                            v_sb[b][:, j, h, c, :], es[idx][:, h, :],
                            start=(idx == 0), stop=(idx == 7),
                            skip_group_check=True)
                    yield
                # normalize: denominators sit at partition 64 of ot[h]
                srow = scpool.tile([1, 2 * QCOLS], F32, tag="srow",
                                   name=f"srow{b}{c}")
                for h in range(2):
                    nc.vector.tensor_copy(
                        srow[0:1, QCOLS * h:QCOLS * (h + 1)], ot[h][64:65, :])
                rr = scpool.tile([1, 2 * QCOLS], F32, tag="rr",
                                 name=f"rr{b}{c}")
                nc.vector.reciprocal_approx_fast(rr[:], srow[:])
                rb = [scpool.tile([64, QCOLS], F32, tag=f"rb{h}",
                                  name=f"rb{b}{c}{h}") for h in range(2)]
                for h in range(2):
                    nc.gpsimd.partition_broadcast(
                        rb[h][:], rr[0:1, QCOLS * h:QCOLS * (h + 1)])
                for h in range(2):
                    nc.vector.tensor_tensor(
                        ocat[b][c][64 * h:64 * h + 64, :], ot[h][0:64, :],
                        rb[h][:], ALU.mult)

            # ---- program ----
            a, v = nc.scalar, nc.vector
            # upfront: all of batch-0 QKV and V^T for both batches
            for c in range(3):
                unit_k(0, c, 0, a)()
                unit_k(0, c, 1, v)()
                unit_q(0, c)()
            for pc in range(8):
                unit_vt(0, pc)()

            # P0 scores with b1 V^T interleaved (spare ot-tag psum slots)
            es = []
            sg = scores_stream(0, 0, es)
            for step in range(8):
                next(sg)
                unit_vt(1, step, pool_tag="ot")()
            av_prev = av_stream(0, 0, es)

            for (b, c) in ((0, 1), (0, 2)):
                es = []
                sg = scores_stream(b, c, es)
                for _ in range(8):
                    next(sg)
                    next(av_prev, None)
                for _ in av_prev:
                    pass
                av_prev = av_stream(b, c, es)

            # mid block: AV(P2) interleaved with dense b1 QKV
            units_mid = [unit_k(1, 0, 0, a), unit_k(1, 0, 1, v), unit_q(1, 0),
                         unit_k(1, 1, 0, a), unit_k(1, 1, 1, v), unit_q(1, 1),
                         unit_k(1, 2, 0, a), unit_k(1, 2, 1, v), unit_q(1, 2)]
            for i in range(8):
                next(av_prev, None)
                units_mid[i]()
            units_mid[8]()
            for _ in av_prev:
                pass

            # P3 scores with proj(b0) interleaved (ot-tag slots)
            es = []
            sg = scores_stream(1, 0, es)
            for step in range(8):
                next(sg)
                if step < 3:
                    unit_proj(0, step, pool_tag="ot")()
            av_prev = av_stream(1, 0, es)

            for (b, c) in ((1, 1), (1, 2)):
                es = []
                sg = scores_stream(b, c, es)
                for _ in range(8):
                    next(sg)
                    next(av_prev, None)
                for _ in av_prev:
                    pass
                av_prev = av_stream(b, c, es)
            for _ in av_prev:
                pass
            for m in range(3):
                unit_proj(1, m)()

    nc.compile()
    return nc


def _build_inputs(x, w_qkv, b_qkv, w_proj, b_proj, attn_idx):
    bf = ml_dtypes.bfloat16
    x = np.asarray(x, np.float32)
    w_qkv = np.asarray(w_qkv, np.float32)
    b_qkv = np.asarray(b_qkv, np.float32)
    w_proj = np.asarray(w_proj, np.float32)
    b_proj = np.asarray(b_proj, np.float32)
    attn_idx = np.asarray(attn_idx).astype(np.int64)

    wqkvT = np.ascontiguousarray(w_qkv.T)  # [in_c, out_c]; q 0:384 k 384:768 v 768:
    blocks = []
    for c in range(3):
        blocks.append(wqkvT[:, C + 128 * c:C + 128 * (c + 1)])          # K(c)
        blocks.append(wqkvT[:, 128 * c:128 * (c + 1)] * SCALE)          # Q(c)
    blocks.append(wqkvT[:, 2 * C:])                                     # V
    wperm = np.ascontiguousarray(np.concatenate(blocks, axis=1)).astype(bf)

    bq = np.ascontiguousarray(
        (b_qkv[:C] * SCALE).reshape(3, 128).T).astype(np.float32)
    bp = np.ascontiguousarray(
        (b_proj + w_proj @ b_qkv[2 * C:]).reshape(3, 128).T).astype(np.float32)
    wprojT = np.ascontiguousarray(w_proj.T).astype(bf)

    in_maps = []
    for i in range(NCORES):
        sr = slab_rows_for_core(i)
        slab = np.zeros((B, C, SLAB, WW), np.float32)
        for s, r in enumerate(sr):
            if r != PAD:
                slab[:, :, s, :] = x[:, :, r, :]
        slab = np.ascontiguousarray(slab.reshape(B, C, SCOLS)).astype(bf)

        # per-pair row -> slab slot lookup (first occurrence in the band)
        row2slot = np.full((4, HH), -1, np.int64)
        for t in range(4):
            for s in range(2 * t + 9, 2 * t - 1, -1):
                if sr[s] != PAD:
                    row2slot[t, sr[s]] = s
        q0 = 8 * i * WW
        aidx = attn_idx[q0:q0 + QCOLS]          # [512, 33]
        t_of_q = np.arange(QCOLS) // 128
        ar = aidx // WW
        ac = aidx % WW
        slot = row2slot[t_of_q[:, None], ar]
        assert (slot >= 0).all(), f"core {i}: target row outside band"
        lidx = slot * 64 + ac                    # local key position
        j = lidx // 128
        kin = lidx % 128
        qq = np.repeat(np.arange(QCOLS), aidx.shape[1])
        jf = j.ravel()
        col = (np.asarray(OFF1)[jf]
               + (qq - 128 * np.asarray(T0_J)[jf]))
        mask = np.zeros((128, 2, MASKW), np.float32)
        mask[kin.ravel(), 0, col] = 1.0
        mask[:, 1, :] = mask[:, 0, :]
        # every query must have exactly 33 targets in its band
        assert int(mask[:, 0, :].sum()) == QCOLS * aidx.shape[1], f"core {i}"

        bd = np.zeros((2, 128), np.float32)
        bd[0, 0:64] = 1.0
        bd[1, 64:128] = 1.0
        in_maps.append({
            "xs": slab,
            "wqkvT": wperm,
            "bq": bq,
            "wprojT": wprojT,
            "bp": bp,
            "mask": np.ascontiguousarray(mask).astype(bf),
            "bd": bd.astype(bf),
        })
    return in_maps


def kernel(x, w_qkv, b_qkv, w_proj, b_proj, attn_idx):
    global LAST_EXEC_NS, LAST_TRACE
    _register_ntff_hook()
    if "graph" not in _NC_CACHE:
        _NC_CACHE["graph"] = build_graph()
    nc = _NC_CACHE["graph"]
    in_maps = _build_inputs(x, w_qkv, b_qkv, w_proj, b_proj, attn_idx)
    trace = bool(int(os.environ.get("BASSK_TRACE", "0")))
    res = run_bass_kernel_spmd(nc, in_maps, core_ids=list(range(NCORES)),
                               trace=trace)
    LAST_EXEC_NS = res.exec_time_ns
    if res.instructions_and_trace is not None:
        LAST_TRACE = res.instructions_and_trace[1]
    out = np.empty((B, C, HH, WW), np.float32)
    for i in range(NCORES):
        o = res.results[i]["out"].reshape(B, C, ROWS, WW)
        out[:, :, 8 * i:8 * i + ROWS, :] = o
    return out
